# revision 7
# baseline (speedup 1.0000x reference)
"""ChannelTimeAttention Trainium2 kernel.

Reference computation (per (b, c) pair, all independent):
    pooled = AdaptiveAvgPool(x[b, :, c]) -> [t, 8*8]      (7x7 block means)
    q = pooled @ Wq + bq ; k = pooled @ Wk + bk           [t, 32]
    att = softmax(q @ k.T / sqrt(t))                      [t, t]
    out[b, :, c] = att @ x[b, :, c].reshape(t, h*w)

Sharding: data-parallel over b — one batch element per NeuronCore (8 cores).
Each core streams its x slice [t=16, c=64, h=56, w=56] through SBUF once in
8 "packs" of 8 channels, with partition layout (t*8 + c_local).  Per pack:
  DVE two-stage strided reduce  -> pooled sums [128, 64]
  PE  transpose + 2 matmuls     -> q^T, k^T [32, 128]
  PE  full 128x128 cross-score matmul + additive block-diag mask + softmax
  PE  transpose(att) -> block-diagonal lhsT, then att @ v in 7 N=448 chunks
  DMA out.
1/49 (pool mean), 1/sqrt(16) (score scale) are folded into Wq/bq/Wk on host.

DMA-stream schedule (the critical design point, target_regime=memory):
Measured on this part: ONE DMA stream (one FIFO queue) sustains only ~215
GB/s HBM; ~2+ concurrent streams saturate (~358 GB/s).  Each queue is FIFO,
and the 16 SDMA engines round-robin between queues with pending work, so
the share per direction tracks the number of active queues per direction.
  - inputs: even packs on nc.sync, odd packs on nc.scalar (the two HWDGE
    rings).  Two streams saturate HBM during the lead-in, packs complete
    pairwise in order, and compute starts at ~12us.
  - outputs: all on the gpsimd SWDGE queue.  In steady state input:output
    queue share is 2:1, which front-loads the input stream (good: the last
    pack's compute+output is the tail) and the output backlog drains at
    full single-stream rate after inputs finish.
The fp32->fp32r rounding of v for the PE (required by the BIR verifier for
fp32r matmuls; a bitcast of the raw f32 bits is rejected) runs on the
otherwise-idle GpSimd engine, off the DVE/ACT critical path.
"""

import numpy as np

B, T, C, H, W = 8, 16, 64, 56, 56
DS = 8
DIN = DS * DS  # 64
DOUT = 32
HW = H * W  # 3136
CG = 8  # channels per pack
NPACK = C // CG  # 8
P = CG * T  # 128 partitions
NCH = 7  # output free-dim chunks per pack
CHN = HW // NCH  # 448
N_CORES = 8
MASK_NEG = -30.0


def _build_nc():
    import concourse.bacc as bacc
    import concourse.tile as tile
    from concourse import mybir
    from contextlib import ExitStack

    f32 = mybir.dt.float32
    f32r = mybir.dt.float32r
    # Bacc (not raw Bass): its compile() runs generate_event_semaphores /
    # move_matmul_waits_to_ldweights, which legalize multi-wait instructions
    # down to the 1-sync-wait-per-instruction TRN2 codegen limit.
    nc = bacc.Bacc(trn_type="TRN2", num_swdge_queues=2)

    x_h = nc.dram_tensor("x", [T, C, H, W], f32, kind="ExternalInput")
    # all small constants packed into ONE [128, 194] array (one DMA with
    # >=512B per-partition rows — six separate tiny DMAs cost ~25us of
    # latency-bound sub-512B descriptors):
    #   cols 0:128   mask, 128:160 wq (rows 0:64), 160:192 wk (rows 0:64),
    #   col 192 bq (rows 0:32), col 193 bk (rows 0:32)
    cn_h = nc.dram_tensor("consts", [P, 194], f32, kind="ExternalInput")
    out_h = nc.dram_tensor("out", [T, C, H, W], f32, kind="ExternalOutput")

    X = mybir.AxisListType.X
    Exp = mybir.ActivationFunctionType.Exp

    with ExitStack() as ctx:
        tc = ctx.enter_context(tile.TileContext(nc))
        singles = ctx.enter_context(tc.tile_pool(name="singles", bufs=1))
        # bufs=NPACK: every v-DMA writes a fresh slot (all 8 issued up-front)
        vpool = ctx.enter_context(tc.tile_pool(name="vpool", bufs=NPACK))
        vrpool = ctx.enter_context(tc.tile_pool(name="vrpool", bufs=3))
        opool = ctx.enter_context(tc.tile_pool(name="opool", bufs=3))
        small = ctx.enter_context(tc.tile_pool(name="small", bufs=2))
        attpool = ctx.enter_context(tc.tile_pool(name="attpool", bufs=3))
        psA = ctx.enter_context(tc.tile_pool(name="psA", bufs=1, space="PSUM"))
        psB = ctx.enter_context(tc.tile_pool(name="psB", bufs=3, space="PSUM"))

        consts = singles.tile([P, 194], f32)
        # consts ride the scalar/ACT HWDGE ring FIRST (odd input packs come
        # later on the same ring; this is tiny).
        nc.scalar.dma_start(out=consts, in_=cn_h[:])
        mask = consts[:, 0:128]
        wq = consts[0:DIN, 128:160]
        wk = consts[0:DIN, 160:192]
        bq = consts[0:DOUT, 192:193]
        bk = consts[0:DOUT, 193:194]
        ident = singles.tile([P, P], f32)

        x_ap = x_h[:]
        out_ap = out_h[:]

        # All 8 input DMAs issued up-front, alternating the two HWDGE rings:
        # each ring is FIFO, so packs complete pairwise in order, and two
        # concurrent streams are enough to cover HBM latency (a single
        # stream measured only ~215 GB/s; two ~saturate).
        v_tiles = []
        for p in range(NPACK):
            c0 = p * CG
            # v[(t*8 + c_l), h*w] = x[t, c0+c_l, h, w]  — t-MAJOR partition
            # order, so the DMA walks DRAM nearly sequentially (100KB runs).
            # (Keep each DMA full-128-partition — 64-partition halves run at
            # half port bandwidth.)
            v = vpool.tile([P, HW], f32, tag="v")
            src = x_ap[:, c0 : c0 + CG, :, :].rearrange("t c h w -> t c (h w)")
            eng = nc.sync if p % 2 == 0 else nc.scalar
            eng.dma_start(out=v[:], in_=src)
            v_tiles.append(v)

        # identity built on-chip (gpsimd memset + affine_select) — no DMA,
        # ready within a few us of kernel start
        from concourse.masks import make_identity

        make_identity(nc, ident[:])

        # Two-stage software pipeline: stage 1 (pool -> q/k -> scores ->
        # softmax -> att^T, plus the fp32r rounding of v) for pack p is
        # emitted BEFORE stage 2 (att @ v -> out DMA) of pack p-1, so the
        # next pack's DVE/ACT/GpSimd work is prioritized ahead of the
        # previous pack's PSUM evacuation and the per-pack cross-engine
        # dependency cycle spans two packs instead of one.
        stage2 = []  # (pack_idx, v_mm, attT)

        def emit_stage1(p):
            v = v_tiles[p]
            # round v to fp32r for the PE — on GpSimd, which is otherwise
            # idle, keeping ACT/DVE free for the softmax chain + evacuation
            v_mm = vrpool.tile([P, HW], f32r, tag="vr")
            nc.gpsimd.tensor_copy(out=v_mm, in_=v)

            # ---- adaptive avg pool (sum; /49 folded into weights) ----
            tmp = small.tile([P, H, DS], f32, tag="tmp")
            nc.vector.reduce_sum(
                out=tmp[:],
                in_=v[:].rearrange("p (h j vv) -> p h j vv", h=H, j=DS, vv=7),
                axis=X,
            )
            pooled = small.tile([P, DS, DS], f32, tag="pooled")
            nc.vector.reduce_sum(
                out=pooled[:],
                in_=tmp[:].rearrange("p (i u) j -> p i j u", i=DS, u=7),
                axis=X,
            )

            # ---- pooled^T via PE so q/k matmuls contract over d_in ----
            pooledT_ps = psA.tile([DIN, P], f32, tag="pooledT_ps")
            nc.tensor.transpose(
                pooledT_ps, pooled[:].rearrange("p i j -> p (i j)"), ident
            )
            pooledT = small.tile([DIN, P], f32, tag="pooledT")
            nc.scalar.copy(pooledT, pooledT_ps)

            # ---- q^T, k^T [32, 128] ----
            qT_ps = psA.tile([DOUT, P], f32, tag="qT_ps")
            nc.tensor.matmul(qT_ps, lhsT=wq, rhs=pooledT, start=True, stop=True)
            kT_ps = psA.tile([DOUT, P], f32, tag="kT_ps")
            nc.tensor.matmul(kT_ps, lhsT=wk, rhs=pooledT, start=True, stop=True)
            qT = small.tile([DOUT, P], f32, tag="qT")
            nc.vector.tensor_scalar_add(out=qT, in0=qT_ps, scalar1=bq)
            kT = small.tile([DOUT, P], f32, tag="kT")
            nc.vector.tensor_scalar_add(out=kT, in0=kT_ps, scalar1=bk)

            # ---- full cross scores [128, 128]; only diag blocks survive mask
            sc_ps = psA.tile([P, P], f32, tag="sc_ps")
            nc.tensor.matmul(sc_ps, lhsT=qT, rhs=kT, start=True, stop=True)
            scm = small.tile([P, P], f32, tag="scm")
            nc.vector.tensor_add(out=scm, in0=sc_ps, in1=mask)

            # ---- softmax along free dim ----
            negm = small.tile([P, 1], f32, tag="negm")
            nc.vector.reduce_max(out=negm, in_=scm, axis=X, negate=True)
            e = small.tile([P, P], f32, tag="e")
            ssum = small.tile([P, 1], f32, tag="ssum")
            nc.scalar.activation(
                out=e, in_=scm, func=Exp, bias=negm, scale=1.0, accum_out=ssum
            )
            rinv = small.tile([P, 1], f32, tag="rinv")
            nc.vector.reciprocal(rinv, ssum)
            att = small.tile([P, P], f32, tag="att")
            nc.vector.tensor_scalar_mul(out=att, in0=e, scalar1=rinv)

            # ---- att^T (block-diagonal) becomes the stationary operand ----
            attT_ps = psA.tile([P, P], f32, tag="attT_ps")
            nc.tensor.transpose(attT_ps, att, ident)
            attT = attpool.tile([P, P], f32r, tag="attT")
            nc.scalar.copy(attT, attT_ps)
            stage2.append((p, v_mm, attT))

        def emit_stage2(p, v_mm, attT):
            c0 = p * CG
            o = opool.tile([P, HW], f32, tag="o")
            # claim the o slot with a cheap DVE op: it absorbs the WAR wait
            # on the out-DMA that previously read this slot
            nc.vector.memset(o[:, 0:1], 0.0)
            for ch in range(NCH):
                sl = slice(ch * CHN, (ch + 1) * CHN)
                ops = psB.tile([P, CHN], f32, tag="ochunk")
                nc.tensor.matmul(
                    ops, lhsT=attT[:], rhs=v_mm[:, sl], start=True, stop=True
                )
                # PSUM->SBUF evacuation: mostly ACT (DVE carries the pool
                # reduces + softmax elementwise)
                if ch == 3:
                    nc.vector.tensor_copy(out=o[:, sl], in_=ops)
                else:
                    nc.scalar.copy(out=o[:, sl], in_=ops)

            # outs all ride the gpsimd SWDGE queue, FIFO in pack order;
            # t-major order writes DRAM nearly sequentially as well
            dst = out_ap[:, c0 : c0 + CG, :, :].rearrange("t c h w -> t c (h w)")
            nc.gpsimd.dma_start(out=dst, in_=o[:])

        for p in range(NPACK):
            emit_stage1(p)
            if p >= 1:
                emit_stage2(*stage2[p - 1])
        emit_stage2(*stage2[NPACK - 1])

    nc.compile()
    return nc


def _host_consts(Wq, bq, Wk, bk):
    # fold pool-mean 1/49 into both weight mats; fold score 1/sqrt(t)=1/4
    # into the q side (weights AND bias)
    wq_eff = (Wq / (49.0 * 4.0)).astype(np.float32)
    bq_eff = (bq / 4.0).astype(np.float32)
    wk_eff = (Wk / 49.0).astype(np.float32)
    bk_eff = bk.astype(np.float32)
    # t-major partition order: row i = (t=i//8, c=i%8); attention pairs
    # (i, j) belong to the same channel iff i%8 == j%8
    idx = np.arange(P)
    same_c = np.equal.outer(idx % CG, idx % CG)
    mask = np.where(same_c, 0.0, MASK_NEG).astype(np.float32)
    consts = np.zeros((P, 194), dtype=np.float32)
    consts[:, 0:128] = mask
    consts[0:DIN, 128:160] = wq_eff
    consts[0:DIN, 160:192] = wk_eff
    consts[0:DOUT, 192] = bq_eff
    consts[0:DOUT, 193] = bk_eff
    return consts


def kernel(x, Wq, bq, Wk, bk):
    from concourse.bass_utils import run_bass_kernel_spmd

    x = np.ascontiguousarray(x, dtype=np.float32)
    consts = _host_consts(Wq, bq, Wk, bk)

    nc = _build_nc()
    in_maps = [{"x": x[i], "consts": consts} for i in range(N_CORES)]
    res = run_bass_kernel_spmd(nc, in_maps, core_ids=list(range(N_CORES)))
    global LAST_RUN
    LAST_RUN = res
    out = np.stack([r["out"] for r in res.results], axis=0)
    return out


LAST_RUN = None


# revision 9
# speedup vs baseline: 1.4201x; 1.4201x over previous
"""ChannelTimeAttention Trainium2 kernel.

Reference computation (per (b, c) pair, all independent):
    pooled = AdaptiveAvgPool(x[b, :, c]) -> [t, 8*8]      (7x7 block means)
    q = pooled @ Wq + bq ; k = pooled @ Wk + bk           [t, 32]
    att = softmax(q @ k.T / sqrt(t))                      [t, t]
    out[b, :, c] = att @ x[b, :, c].reshape(t, h*w)

Sharding: data-parallel over b — one batch element per NeuronCore (8 cores).
Each core streams its x slice [t=16, c=64, h=56, w=56] through SBUF once in
8 "packs" of 8 channels, partition layout (t*8 + c_local).  Per pack:
  DVE one-shot 2-axis strided reduce -> pooled sums [128, 64]
  PE  transpose + 2 matmuls          -> q^T, k^T [32, 128]  (f32)
  PE  full 128x128 cross-score matmul + additive block-diag mask
  softmax WITHOUT final normalize: e = exp(s - max), sum via ACT accumulator;
  PE  transpose(e) -> block-diagonal lhsT (bf16), e^T @ v in 7 N=448 chunks,
  the 1/sum normalization is folded into the PSUM->SBUF evacuation scale.
1/49 (pool mean), 1/sqrt(16) (score scale) are folded into Wq/bq/Wk on host.

The value path (v and e^T) runs in bf16: the input DMA casts f32->bf16
inline (SWDGE supports dtype conversion; HBM read traffic is unchanged,
SBUF footprint halves, and no separate fp32->fp32r rounding pass is needed
for the PE).  The q/k/score path stays f32 from the f32 HBM read... (the
pooling reduce reads the bf16 v; pooled sums accumulate in f32).  Expected
extra error ~1e-3 relative, well inside the 2e-2 gate.

DMA-stream schedule (the critical design point, target_regime=memory):
Measured here: ONE stream (any queue) sustains ~215 GB/s; the two HWDGE
rings share one TPB-level descriptor generator (~240 GB/s combined); and
read+write streams overlap almost additively (mixed phases sustain
~1.0-1.17 of nominal).  So:
  - ALL 8 input pack DMAs ride the SWDGE queue (nc.gpsimd), issued
    up-front: FIFO, so pack p completes before pack p+1 at the full
    single-stream rate (~7.5us per 1.6MB pack).
  - outputs alternate the two HWDGE rings (nc.sync / nc.scalar), which
    need ~180 GB/s average and overlap the read stream.
  - compute cadence (~5.5-6us/pack max engine) sits under the 7.5us input
    cadence so the output stream is never compute-starved.
"""

import numpy as np

B, T, C, H, W = 8, 16, 64, 56, 56
DS = 8
DIN = DS * DS  # 64
DOUT = 32
HW = H * W  # 3136
CG = 8  # channels per pack
NPACK = C // CG  # 8
P = CG * T  # 128 partitions
NCH = 7  # output free-dim chunks per pack
CHN = HW // NCH  # 448
N_CORES = 8
MASK_NEG = -30.0


def _build_nc():
    import concourse.bacc as bacc
    import concourse.tile as tile
    from concourse import mybir
    from contextlib import ExitStack

    f32 = mybir.dt.float32
    bf16 = mybir.dt.bfloat16
    nc = bacc.Bacc(trn_type="TRN2", num_swdge_queues=2)

    x_h = nc.dram_tensor("x", [T, C, H, W], f32, kind="ExternalInput")
    # all small constants packed into ONE [128, 194] array:
    #   cols 0:128   mask, 128:160 wq (rows 0:64), 160:192 wk (rows 0:64),
    #   col 192 bq (rows 0:32), col 193 bk (rows 0:32)
    cn_h = nc.dram_tensor("consts", [P, 194], f32, kind="ExternalInput")
    out_h = nc.dram_tensor("out", [T, C, H, W], f32, kind="ExternalOutput")

    X = mybir.AxisListType.X
    XY = mybir.AxisListType.XY
    Exp = mybir.ActivationFunctionType.Exp
    Copy = mybir.ActivationFunctionType.Copy

    with ExitStack() as ctx:
        tc = ctx.enter_context(tile.TileContext(nc))
        singles = ctx.enter_context(tc.tile_pool(name="singles", bufs=1))
        # bufs=NPACK: every v-DMA writes a fresh slot (all 8 issued up-front)
        vpool = ctx.enter_context(tc.tile_pool(name="vpool", bufs=NPACK))
        opool = ctx.enter_context(tc.tile_pool(name="opool", bufs=4))
        small = ctx.enter_context(tc.tile_pool(name="small", bufs=2))
        attpool = ctx.enter_context(tc.tile_pool(name="attpool", bufs=3))
        psA = ctx.enter_context(tc.tile_pool(name="psA", bufs=1, space="PSUM"))
        psB = ctx.enter_context(tc.tile_pool(name="psB", bufs=4, space="PSUM"))

        consts = singles.tile([P, 194], f32)
        # consts ride the sync HWDGE ring first (even output packs come
        # later on the same ring; this is tiny).
        nc.sync.dma_start(out=consts, in_=cn_h[:])
        mask = consts[:, 0:128]
        wq = consts[0:DIN, 128:160]
        wk = consts[0:DIN, 160:192]
        bq = consts[0:DOUT, 192:193]
        bk = consts[0:DOUT, 193:194]
        ident = singles.tile([P, P], f32)

        # identity built on-chip (gpsimd memset + affine_select) — emitted
        # BEFORE the input DMAs so it isn't queued behind the Q7 descriptor
        # generation for 8 big transfers on the same engine.
        from concourse.masks import make_identity

        make_identity(nc, ident[:])

        x_ap = x_h[:]
        out_ap = out_h[:]

        # All 8 input DMAs issued up-front on the SWDGE queue (nc.gpsimd):
        # FIFO execution means pack p completes before pack p+1 starts, each
        # at the full single-stream rate.  The f32->bf16 cast rides the DMA.
        v_tiles = []
        for p in range(NPACK):
            c0 = p * CG
            # v[(t*8 + c_l), h*w] = x[t, c0+c_l, h, w]  — t-MAJOR partition
            # order, so the DMA walks DRAM nearly sequentially (100KB runs).
            v = vpool.tile([P, HW], bf16, tag="v")
            src = x_ap[:, c0 : c0 + CG, :, :].rearrange("t c h w -> t c (h w)")
            nc.gpsimd.dma_start(out=v[:], in_=src)
            v_tiles.append(v)

        # Two-stage software pipeline: stage 1 (pool -> q/k -> scores ->
        # softmax numerator -> e^T) for pack p is emitted BEFORE stage 2
        # (e^T @ v -> scaled evac -> out DMA) of pack p-1, so the next
        # pack's DVE/ACT work is prioritized ahead of the previous pack's
        # PSUM evacuation and the per-pack cross-engine dependency cycle
        # spans two packs instead of one.
        stage2 = []  # (pack_idx, v, eT, rinv)

        def emit_stage1(p):
            v = v_tiles[p]

            # ---- adaptive avg pool (sum; /49 folded into weights) ----
            # one 2-axis reduce: [p, i, j, u, vv] -> sum over (u, vv);
            # bf16 in, f32 accumulate/out
            pooled = small.tile([P, DS, DS], f32, tag="pooled")
            nc.vector.reduce_sum(
                out=pooled[:],
                in_=v[:].rearrange(
                    "p (i u j vv) -> p i j u vv", i=DS, u=7, j=DS, vv=7
                ),
                axis=XY,
            )

            # ---- pooled^T via PE so q/k matmuls contract over d_in ----
            pooledT_ps = psA.tile([DIN, P], f32, tag="pooledT_ps")
            nc.tensor.transpose(
                pooledT_ps, pooled[:].rearrange("p i j -> p (i j)"), ident
            )
            pooledT = small.tile([DIN, P], f32, tag="pooledT")
            nc.scalar.copy(pooledT, pooledT_ps)

            # ---- q^T, k^T [32, 128], sharing one PSUM bank ----
            qkT_ps = psA.tile([DOUT, 2 * P], f32, tag="qkT_ps")
            nc.tensor.matmul(
                qkT_ps[:, 0:P], lhsT=wq, rhs=pooledT, start=True, stop=True
            )
            nc.tensor.matmul(
                qkT_ps[:, P : 2 * P], lhsT=wk, rhs=pooledT, start=True, stop=True
            )
            qT = small.tile([DOUT, P], f32, tag="qT")
            nc.vector.tensor_scalar_add(out=qT, in0=qkT_ps[:, 0:P], scalar1=bq)
            kT = small.tile([DOUT, P], f32, tag="kT")
            nc.vector.tensor_scalar_add(
                out=kT, in0=qkT_ps[:, P : 2 * P], scalar1=bk
            )

            # ---- full cross scores [128, 128]; only diag blocks survive mask
            sc_ps = psA.tile([P, P], f32, tag="sc_ps")
            nc.tensor.matmul(sc_ps, lhsT=qT, rhs=kT, start=True, stop=True)
            scm = small.tile([P, P], f32, tag="scm")
            nc.vector.tensor_add(out=scm, in0=sc_ps, in1=mask)

            # ---- softmax numerator + row sums; 1/sum folded into evac ----
            negm = small.tile([P, 1], f32, tag="negm")
            nc.vector.reduce_max(out=negm, in_=scm, axis=X, negate=True)
            e = small.tile([P, P], f32, tag="e")
            ssum = small.tile([P, 1], f32, tag="ssum")
            nc.scalar.activation(
                out=e, in_=scm, func=Exp, bias=negm, scale=1.0, accum_out=ssum
            )
            rinv = small.tile([P, 1], f32, tag="rinv")
            nc.vector.reciprocal(rinv, ssum)

            # ---- e^T (block-diagonal) becomes the stationary operand ----
            eT_ps = psA.tile([P, P], f32, tag="eT_ps")
            nc.tensor.transpose(eT_ps, e, ident)
            eT = attpool.tile([P, P], bf16, tag="eT")
            nc.scalar.copy(eT, eT_ps)
            stage2.append((p, v, eT, rinv))

        def emit_stage2(p, v, eT, rinv):
            c0 = p * CG
            o = opool.tile([P, HW], f32, tag="o")
            # claim the o slot with a cheap DVE op: it absorbs the WAR wait
            # on the out-DMA that previously read this slot
            nc.vector.memset(o[:, 0:1], 0.0)
            for ch in range(NCH):
                sl = slice(ch * CHN, (ch + 1) * CHN)
                ops = psB.tile([P, CHN], f32, tag="ochunk")
                nc.tensor.matmul(
                    ops, lhsT=eT[:], rhs=v[:, sl], start=True, stop=True
                )
                # PSUM->SBUF evacuation scaled by 1/rowsum (the softmax
                # normalization); ACT takes most chunks (DVE carries the
                # pool reduce + score/softmax elementwise)
                if ch == 3:
                    nc.vector.tensor_scalar_mul(
                        out=o[:, sl], in0=ops, scalar1=rinv
                    )
                else:
                    nc.scalar.activation(
                        out=o[:, sl], in_=ops, func=Copy, scale=rinv
                    )

            # outs alternate the two HWDGE rings, FIFO per ring in pack
            # order; t-major order writes DRAM nearly sequentially as well
            dst = out_ap[:, c0 : c0 + CG, :, :].rearrange("t c h w -> t c (h w)")
            eng = nc.sync if p % 2 == 0 else nc.scalar
            eng.dma_start(out=dst, in_=o[:])

        for p in range(NPACK):
            emit_stage1(p)
            if p >= 1:
                emit_stage2(*stage2[p - 1])
        emit_stage2(*stage2[NPACK - 1])

    nc.compile()
    return nc


def _host_consts(Wq, bq, Wk, bk):
    # fold pool-mean 1/49 into both weight mats; fold score 1/sqrt(t)=1/4
    # into the q side (weights AND bias)
    wq_eff = (Wq / (49.0 * 4.0)).astype(np.float32)
    bq_eff = (bq / 4.0).astype(np.float32)
    wk_eff = (Wk / 49.0).astype(np.float32)
    bk_eff = bk.astype(np.float32)
    # t-major partition order: row i = (t=i//8, c=i%8); attention pairs
    # (i, j) belong to the same channel iff i%8 == j%8
    idx = np.arange(P)
    same_c = np.equal.outer(idx % CG, idx % CG)
    mask = np.where(same_c, 0.0, MASK_NEG).astype(np.float32)
    consts = np.zeros((P, 194), dtype=np.float32)
    consts[:, 0:128] = mask
    consts[0:DIN, 128:160] = wq_eff
    consts[0:DIN, 160:192] = wk_eff
    consts[0:DOUT, 192] = bq_eff
    consts[0:DOUT, 193] = bk_eff
    return consts


def kernel(x, Wq, bq, Wk, bk):
    from concourse.bass_utils import run_bass_kernel_spmd

    x = np.ascontiguousarray(x, dtype=np.float32)
    consts = _host_consts(Wq, bq, Wk, bk)

    nc = _build_nc()
    in_maps = [{"x": x[i], "consts": consts} for i in range(N_CORES)]
    res = run_bass_kernel_spmd(nc, in_maps, core_ids=list(range(N_CORES)))
    global LAST_RUN
    LAST_RUN = res
    out = np.stack([r["out"] for r in res.results], axis=0)
    return out


LAST_RUN = None


# revision 11
# speedup vs baseline: 1.5122x; 1.0649x over previous
"""ChannelTimeAttention Trainium2 kernel.

Reference computation (per (b, c) pair, all independent):
    pooled = AdaptiveAvgPool(x[b, :, c]) -> [t, 8*8]      (7x7 block means)
    q = pooled @ Wq + bq ; k = pooled @ Wk + bk           [t, 32]
    att = softmax(q @ k.T / sqrt(t))                      [t, t]
    out[b, :, c] = att @ x[b, :, c].reshape(t, h*w)

Sharding: data-parallel over b — one batch element per NeuronCore (8 cores).
Each core streams its x slice [t=16, c=64, h=56, w=56] through SBUF once in
8 "packs" of 8 channels, partition layout (t*8 + c_local).  Per pack:
  DVE one-shot 2-axis strided reduce -> pooled sums [128, 64]
  PE  transpose + 2 matmuls          -> q^T, k^T [32, 128]  (f32)
  PE  cross-score matmul with the block-diagonal attention mask FOLDED IN:
      additive mask M = -30*(1 - same_channel) is rank-9
      (M = -30*ones + 30*sum_c a_c a_c^T), so qT/kT get 9 constant extra
      contraction rows instead of a separate [128,128] DVE mask add.
  softmax WITHOUT final normalize: e = exp(s - max), sum via ACT accumulator;
  PE  transpose(e) -> block-diagonal lhsT (bf16), e^T @ v in 7 N=448 chunks,
  the 1/sum normalization is folded into the PSUM->SBUF evacuation scale,
  and each half of the evacuated output is DMA'd out as soon as it's ready.
1/49 (pool mean), 1/sqrt(16) (score scale) are folded into Wq/bq/Wk on host.

The value path (v and e^T) runs in bf16: the input DMA casts f32->bf16
inline (SWDGE supports dtype conversion; HBM read traffic unchanged, SBUF
write side halves — which nearly 1.5x'd the input stream rate, 215->319
GB/s — and the PE gets its fast-path dtype with no rounding pass).
Expected extra error ~1e-3 relative, inside the 2e-2 gate.

DMA-stream schedule (target_regime=memory):
  - ALL 8 input pack DMAs ride the SWDGE queue (nc.gpsimd), issued
    up-front: FIFO, pack p completes before pack p+1, ~4.3us per pack.
  - outputs: TWO half-pack DMAs per pack (cols 0:1792 on nc.sync, cols
    1792:3136 on nc.scalar), each gated only on its own 4 (resp. 3)
    PSUM-evacuation chunks, so both HWDGE rings stream writes that
    overlap the read stream (reads+writes share HBM almost additively).
"""

import numpy as np

B, T, C, H, W = 8, 16, 64, 56, 56
DS = 8
DIN = DS * DS  # 64
DOUT = 32
EXT = DOUT + 9  # 41: q/k plus 9 constant mask rows
HW = H * W  # 3136
CG = 8  # channels per pack
NPACK = C // CG  # 8
P = CG * T  # 128 partitions
NCH = 7  # output free-dim chunks per pack
CHN = HW // NCH  # 448
HALF1 = 4 * CHN  # 1792
N_CORES = 8
MASK_NEG = -30.0


def _build_nc():
    import concourse.bacc as bacc
    import concourse.tile as tile
    from concourse import mybir
    from contextlib import ExitStack

    f32 = mybir.dt.float32
    bf16 = mybir.dt.bfloat16
    nc = bacc.Bacc(trn_type="TRN2", num_swdge_queues=2)

    x_h = nc.dram_tensor("x", [T, C, H, W], f32, kind="ExternalInput")
    # consts [128, 322]:
    #   cols 0:128   qext (rows 32:41: [-30*ones; 30*a_c])
    #   cols 128:160 wq (rows 0:64), 160:192 wk (rows 0:64)
    #   col 192 bq (rows 0:32), col 193 bk (rows 0:32)
    #   cols 194:322 kext (rows 32:41: [ones; a_c])
    cn_h = nc.dram_tensor("consts", [P, 322], f32, kind="ExternalInput")
    out_h = nc.dram_tensor("out", [T, C, H, W], f32, kind="ExternalOutput")

    X = mybir.AxisListType.X
    XY = mybir.AxisListType.XY
    Exp = mybir.ActivationFunctionType.Exp
    Copy = mybir.ActivationFunctionType.Copy

    with ExitStack() as ctx:
        tc = ctx.enter_context(tile.TileContext(nc))
        singles = ctx.enter_context(tc.tile_pool(name="singles", bufs=1))
        vpool = ctx.enter_context(tc.tile_pool(name="vpool", bufs=NPACK))
        opool = ctx.enter_context(tc.tile_pool(name="opool", bufs=6))
        small = ctx.enter_context(tc.tile_pool(name="small", bufs=3))
        attpool = ctx.enter_context(tc.tile_pool(name="attpool", bufs=3))
        psA = ctx.enter_context(tc.tile_pool(name="psA", bufs=1, space="PSUM"))
        psB = ctx.enter_context(tc.tile_pool(name="psB", bufs=6, space="PSUM"))

        consts = singles.tile([P, 322], f32)
        nc.sync.dma_start(out=consts, in_=cn_h[:])
        wq = consts[0:DIN, 128:160]
        wk = consts[0:DIN, 160:192]
        bq = consts[0:DOUT, 192:193]
        bk = consts[0:DOUT, 193:194]
        ident = singles.tile([P, P], f32)

        # persistent q^T/k^T operands [41, 128]: rows 0:32 rewritten per
        # pack (bias add), rows 32:41 filled ONCE with the mask fold rows
        qTt = singles.tile([EXT, P], f32)
        kTt = singles.tile([EXT, P], f32)
        nc.vector.tensor_copy(out=qTt[DOUT:EXT, :], in_=consts[DOUT:EXT, 0:P])
        nc.vector.tensor_copy(
            out=kTt[DOUT:EXT, :], in_=consts[DOUT:EXT, 194:322]
        )

        # identity built on-chip (gpsimd memset + affine_select) — emitted
        # BEFORE the input DMAs so it isn't queued behind the Q7 descriptor
        # generation for 8 big transfers on the same engine.
        from concourse.masks import make_identity

        make_identity(nc, ident[:])

        x_ap = x_h[:]
        out_ap = out_h[:]

        # All 8 input DMAs issued up-front on the SWDGE queue (FIFO).
        v_tiles = []
        for p in range(NPACK):
            c0 = p * CG
            # v[(t*8 + c_l), h*w] = x[t, c0+c_l, h, w]  — t-MAJOR partition
            # order, so the DMA walks DRAM nearly sequentially (100KB runs).
            v = vpool.tile([P, HW], bf16, tag="v")
            src = x_ap[:, c0 : c0 + CG, :, :].rearrange("t c h w -> t c (h w)")
            nc.gpsimd.dma_start(out=v[:], in_=src)
            v_tiles.append(v)

        # Two-stage software pipeline: stage 1 of pack p is emitted before
        # stage 2 of pack p-1.
        stage2 = []  # (pack_idx, v, eT, rinv)

        def emit_stage1(p):
            v = v_tiles[p]

            # ---- adaptive avg pool (sum; /49 folded into weights) ----
            pooled = small.tile([P, DS, DS], f32, tag="pooled")
            nc.vector.reduce_sum(
                out=pooled[:],
                in_=v[:].rearrange(
                    "p (i u j vv) -> p i j u vv", i=DS, u=7, j=DS, vv=7
                ),
                axis=XY,
            )

            # ---- pooled^T, q^T, k^T all through ONE shared PSUM bank ----
            psQK = psA.tile([DIN, 384], f32, tag="psQK")
            nc.tensor.transpose(
                psQK[:, 0:P], pooled[:].rearrange("p i j -> p (i j)"), ident
            )
            pooledT = small.tile([DIN, P], f32, tag="pooledT")
            nc.scalar.copy(pooledT, psQK[:, 0:P])
            nc.tensor.matmul(
                psQK[0:DOUT, 128:256], lhsT=wq, rhs=pooledT, start=True,
                stop=True,
            )
            nc.tensor.matmul(
                psQK[0:DOUT, 256:384], lhsT=wk, rhs=pooledT, start=True,
                stop=True,
            )
            # bias adds write rows 0:32 of the persistent operands (DVE;
            # ACT's Copy func rejects per-partition AP bias)
            nc.vector.tensor_scalar_add(
                out=qTt[0:DOUT, :], in0=psQK[0:DOUT, 128:256], scalar1=bq
            )
            nc.vector.tensor_scalar_add(
                out=kTt[0:DOUT, :], in0=psQK[0:DOUT, 256:384], scalar1=bk
            )

            # ---- scores WITH mask folded in (41-row contraction) ----
            psE = psA.tile([P, 256], f32, tag="psE")
            nc.tensor.matmul(
                psE[:, 0:P], lhsT=qTt[:], rhs=kTt[:], start=True, stop=True
            )

            # ---- softmax numerator straight from PSUM ----
            negm = small.tile([P, 1], f32, tag="negm")
            nc.vector.reduce_max(out=negm, in_=psE[:, 0:P], axis=X, negate=True)
            e = small.tile([P, P], f32, tag="e")
            ssum = small.tile([P, 1], f32, tag="ssum")
            nc.scalar.activation(
                out=e, in_=psE[:, 0:P], func=Exp, bias=negm, scale=1.0,
                accum_out=ssum,
            )
            rinv = small.tile([P, 1], f32, tag="rinv")
            nc.vector.reciprocal(rinv, ssum)

            # ---- e^T (block-diagonal) becomes the stationary operand ----
            nc.tensor.transpose(psE[:, 128:256], e, ident)
            eT = attpool.tile([P, P], bf16, tag="eT")
            nc.scalar.copy(eT, psE[:, 128:256])
            stage2.append((p, v, eT, rinv))

        def emit_stage2(p, v, eT, rinv):
            c0 = p * CG
            o = opool.tile([P, HW], f32, tag="o")
            # claim the o slot with a cheap DVE op: absorbs the WAR wait on
            # the out-DMAs that previously read this slot
            nc.vector.memset(o[:, 0:1], 0.0)
            dst = out_ap[:, c0 : c0 + CG, :, :].rearrange("t c h w -> t c (h w)")
            for ch in range(NCH):
                sl = slice(ch * CHN, (ch + 1) * CHN)
                ops = psB.tile([P, CHN], f32, tag="ochunk")
                nc.tensor.matmul(
                    ops, lhsT=eT[:], rhs=v[:, sl], start=True, stop=True
                )
                # PSUM->SBUF evacuation scaled by 1/rowsum (the softmax
                # normalization), split between DVE and ACT
                if ch in (0, 4):
                    nc.vector.tensor_scalar_mul(
                        out=o[:, sl], in0=ops, scalar1=rinv
                    )
                else:
                    nc.scalar.activation(
                        out=o[:, sl], in_=ops, func=Copy, scale=rinv
                    )
                if ch == 3:
                    # first half (chunks 0-3) ready -> stream it now
                    nc.sync.dma_start(out=dst[:, :, 0:HALF1], in_=o[:, 0:HALF1])
            nc.scalar.dma_start(out=dst[:, :, HALF1:HW], in_=o[:, HALF1:HW])

        for p in range(NPACK):
            emit_stage1(p)
            if p >= 1:
                emit_stage2(*stage2[p - 1])
        emit_stage2(*stage2[NPACK - 1])

    nc.compile()
    return nc


def _host_consts(Wq, bq, Wk, bk):
    # fold pool-mean 1/49 into both weight mats; fold score 1/sqrt(t)=1/4
    # into the q side (weights AND bias)
    wq_eff = (Wq / (49.0 * 4.0)).astype(np.float32)
    bq_eff = (bq / 4.0).astype(np.float32)
    wk_eff = (Wk / 49.0).astype(np.float32)
    bk_eff = bk.astype(np.float32)
    # t-major partition order: row i = (t=i//8, c=i%8); attention pairs
    # (i, j) belong to the same channel iff i%8 == j%8.  The additive mask
    # M = -30*(1-same_c) is rank-9: M = -30*ones + 30*sum_c a_c a_c^T.
    idx = np.arange(P)
    a = np.stack([(idx % CG == c).astype(np.float32) for c in range(CG)])
    qext = np.vstack([MASK_NEG * np.ones((1, P), np.float32), -MASK_NEG * a])
    kext = np.vstack([np.ones((1, P), np.float32), a])
    consts = np.zeros((P, 322), dtype=np.float32)
    consts[DOUT:EXT, 0:P] = qext
    consts[0:DIN, 128:160] = wq_eff
    consts[0:DIN, 160:192] = wk_eff
    consts[0:DOUT, 192] = bq_eff
    consts[0:DOUT, 193] = bk_eff
    consts[DOUT:EXT, 194:322] = kext
    return consts


def kernel(x, Wq, bq, Wk, bk):
    from concourse.bass_utils import run_bass_kernel_spmd

    x = np.ascontiguousarray(x, dtype=np.float32)
    consts = _host_consts(Wq, bq, Wk, bk)

    nc = _build_nc()
    in_maps = [{"x": x[i], "consts": consts} for i in range(N_CORES)]
    res = run_bass_kernel_spmd(nc, in_maps, core_ids=list(range(N_CORES)))
    global LAST_RUN
    LAST_RUN = res
    out = np.stack([r["out"] for r in res.results], axis=0)
    return out


LAST_RUN = None
